# revision 4
# baseline (speedup 1.0000x reference)
"""Trainium2 Bass kernel for BatchedExpertDispatch (MoE routing, top-2, identity experts).

Math: with identity experts, the dispatch->expert->combine round trip reduces to
  combined[n] = hidden_states[n] * (w1[n] + w2[n])
where (w1, w2) are the normalized top-2 softmax weights:
  e2 = exp(x_top2 - x_top1);  w1 = 1/(1+e2);  w2 = e2/(1+e2)
expert_indices are the top-2 argmax indices of the router logits (softmax is
monotonic, so top-k of logits == top-k of probs).

Sharding: data-parallel over the token dim. N=8192 tokens split across 8 cores
(1024 tokens each); routing/combine are token-local, no cross-core traffic.

Per-core device work (all in one SPMD Bass program):
  - router logits [1024, 64] loaded as [128p, 8g*64] (host pre-transposes);
    per 128-token group: DVE max/max_index for top-2, ACT exp, DVE reciprocal.
  - hidden stream: 8 tiles of [128, 4096] f32 (2MB), DMA in (SP HWDGE ring),
    per-token scalar multiply (DVE/ACT alternating), DMA out (ACT HWDGE ring).
"""

import numpy as np

N, H, E, K = 8192, 4096, 64, 2
NCORES = 8
NP = N // NCORES  # tokens per core
P = 128           # SBUF partitions
G = NP // P       # 128-token groups per core

_CACHE = {}


def _ensure_path():
    import sys
    try:
        import concourse.bass  # noqa: F401
    except ImportError:
        sys.path.insert(0, "/opt/trn_rl_repo")


def _legalize_waits(bir_json: bytes) -> bytes:
    """Split multi-wait instructions for toolchains whose ISA structs encode
    only one sync wait: extra waits move to standalone EventSemaphore
    instructions inserted just before, on the same engine (sem-ge waits are
    monotonic, so waiting earlier in program order is equivalent)."""
    import json as _json

    m = _json.loads(bir_json)
    counter = 0
    for fn in m["functions"]:
        for blk in fn["blocks"]:
            new_instructions = []
            for ins in blk["instructions"]:
                si = ins.get("sync_info")
                waits = (si or {}).get("on_wait") or []
                if len(waits) > 1 and ins.get("opcode") != "EventSemaphore":
                    for w in waits[:-1]:
                        counter += 1
                        new_instructions.append({
                            "debug": ins.get("debug", 0),
                            "engine": ins["engine"],
                            "ins": [], "outs": [],
                            "name": f"W-{counter}-{ins['name']}",
                            "opcode": "EventSemaphore",
                            "sync_info": {"on_update": [], "on_wait": [w]},
                        })
                    si["on_wait"] = [waits[-1]]
                new_instructions.append(ins)
            blk["instructions"] = new_instructions
    return _json.dumps(m).encode()


def _patch_compiler():
    """Route every BIR compile through _legalize_waits (native + axon paths)."""
    from concourse import bass_utils, bass2jax

    if getattr(bass_utils, "_wait_legalizer_installed", False):
        return
    orig = bass_utils.compile_bir_kernel

    def patched(bir_json, tmpdir, neff_name="file.neff"):
        return orig(_legalize_waits(bytes(bir_json)), tmpdir, neff_name)

    bass_utils.compile_bir_kernel = patched
    bass2jax.compile_bir_kernel = patched
    bass_utils._wait_legalizer_installed = True


def _build_program():
    import concourse.bass as bass
    import concourse.mybir as mybir
    import concourse.tile as tile

    f32 = mybir.dt.float32
    i32 = mybir.dt.int32
    u32 = mybir.dt.uint32
    Exp = mybir.ActivationFunctionType.Exp

    nc = bass.Bass(
        "TRN2", target_bir_lowering=False, debug=False, num_devices=NCORES
    )

    hs = nc.dram_tensor("hs_in", [NP, H], f32, kind="ExternalInput").ap()
    # router logits pre-rearranged on host to [128, G*E]: (p, g*E+e) <- token g*128+p
    rp = nc.dram_tensor("rp_in", [P, G * E], f32, kind="ExternalInput").ap()
    out_c = nc.dram_tensor("combined_out", [NP, H], f32, kind="ExternalOutput").ap()
    # small outputs stay in the [128, G*K] on-chip layout; host de-interleaves
    out_ei = nc.dram_tensor("ei_out", [P, G * K], i32, kind="ExternalOutput").ap()
    out_rw = nc.dram_tensor("rw_out", [P, G * K], f32, kind="ExternalOutput").ap()

    with tile.TileContext(nc) as tc:
        with tc.tile_pool(name="router", bufs=1) as rpool, \
             tc.tile_pool(name="big", bufs=6) as hpool:
            r = rpool.tile([P, G * E], f32, tag="r")
            nc.sync.dma_start(out=r[:], in_=rp[:])
            ei_sb = rpool.tile([P, G * K], i32, tag="ei")
            rw_sb = rpool.tile([P, G * K], f32, tag="rw")
            scale = rpool.tile([P, G], f32, tag="scale")
            # per-group top-8 values/indices, group g at columns [g*8, g*8+8)
            mx_all = rpool.tile([P, G * 8], f32, tag="mxall")
            mi_all = rpool.tile([P, G * 8], u32, tag="miall")

            for g in range(G):
                x = r[:, g * E:(g + 1) * E]
                nc.vector.max(out=mx_all[:, g * 8:(g + 1) * 8], in_=x)
                nc.vector.max_index(
                    out=mi_all[:, g * 8:(g + 1) * 8],
                    in_max=mx_all[:, g * 8:(g + 1) * 8],
                    in_values=x,
                )

            # batched over all G groups via strided views
            m1 = mx_all[:, 0:G * 8:8]   # [P, G] top-1 logit per group
            m2 = mx_all[:, 1:G * 8:8]   # [P, G] top-2 logit per group
            d = rpool.tile([P, G], f32, tag="d")
            nc.vector.tensor_sub(out=d[:], in0=m2, in1=m1)          # m2-m1 <= 0
            e2 = rpool.tile([P, G], f32, tag="e2")
            nc.scalar.activation(e2[:], d[:], Exp)                  # exp(m2-m1)
            denom = rpool.tile([P, G], f32, tag="denom")
            nc.vector.tensor_scalar_add(denom[:], e2[:], 1.0)       # 1+e2
            w1 = rw_sb[:, 0:G * K:2]
            w2 = rw_sb[:, 1:G * K:2]
            nc.vector.reciprocal(w1, denom[:])                      # w1 = 1/(1+e2)
            nc.vector.tensor_mul(out=w2, in0=e2[:], in1=w1)         # w2 = e2/(1+e2)
            nc.vector.tensor_add(out=scale[:], in0=w1, in1=w2)
            nc.vector.tensor_copy(out=ei_sb[:, 0:G * K:2], in_=mi_all[:, 0:G * 8:8])
            nc.vector.tensor_copy(out=ei_sb[:, 1:G * K:2], in_=mi_all[:, 1:G * 8:8])

            nc.sync.dma_start(out=out_ei[:], in_=ei_sb[:])
            nc.sync.dma_start(out=out_rw[:], in_=rw_sb[:])

            for t in range(G):
                h = hpool.tile([P, H], f32, tag="h")
                nc.sync.dma_start(out=h[:], in_=hs[t * P:(t + 1) * P, :])
                nc.vector.tensor_scalar_mul(h[:], h[:], scale[:, t:t + 1])
                nc.scalar.dma_start(out=out_c[t * P:(t + 1) * P, :], in_=h[:])

    return nc


# test.py hooks: set TRACE=True before calling kernel() to capture a profile.
TRACE = False
LAST_RESULTS = None


def kernel(hidden_states, router_probs, top_k=2, **_unused):
    global LAST_RESULTS
    _ensure_path()
    _patch_compiler()
    from concourse import bass_utils

    assert int(top_k) == K, f"kernel hardcodes top_k={K}, got {top_k}"
    hs = np.ascontiguousarray(np.asarray(hidden_states, dtype=np.float32))
    rp = np.ascontiguousarray(np.asarray(router_probs, dtype=np.float32))
    assert hs.shape == (N, H) and rp.shape == (N, E), (hs.shape, rp.shape)

    if "nc" not in _CACHE:
        _CACHE["nc"] = _build_program()
    nc = _CACHE["nc"]

    in_maps = []
    for c in range(NCORES):
        hs_c = hs[c * NP:(c + 1) * NP]
        rp_c = rp[c * NP:(c + 1) * NP].reshape(G, P, E).transpose(1, 0, 2)
        in_maps.append({
            "hs_in": np.ascontiguousarray(hs_c),
            "rp_in": np.ascontiguousarray(rp_c.reshape(P, G * E)),
        })

    res = bass_utils.run_bass_kernel_spmd(
        nc, in_maps, core_ids=list(range(NCORES)), trace=TRACE
    )
    LAST_RESULTS = res

    combined = np.concatenate([res.results[c]["combined_out"] for c in range(NCORES)], axis=0)
    ei = np.concatenate([
        res.results[c]["ei_out"].reshape(P, G, K).transpose(1, 0, 2).reshape(NP, K)
        for c in range(NCORES)
    ], axis=0)
    rw = np.concatenate([
        res.results[c]["rw_out"].reshape(P, G, K).transpose(1, 0, 2).reshape(NP, K)
        for c in range(NCORES)
    ], axis=0)
    return combined, ei.astype(np.int32, copy=False), rw


# revision 6
# speedup vs baseline: 271.9666x; 271.9666x over previous
"""Trainium2 Bass kernel for BatchedExpertDispatch (MoE routing, top-2, identity experts).

Math: with identity experts, the dispatch->expert->combine round trip reduces to
  combined[n] = hidden_states[n] * (w1[n] + w2[n])
where (w1, w2) are the normalized top-2 softmax weights:
  e2 = exp(x_top2 - x_top1);  w1 = 1/(1+e2);  w2 = e2/(1+e2)
expert_indices are the top-2 argmax indices of the router logits (softmax is
monotonic, so top-k of logits == top-k of probs).

Sharding: data-parallel over the token dim. N=8192 tokens split across 8 cores
(1024 tokens each); routing/combine are token-local, no cross-core traffic.

Per-core device work (all in one SPMD Bass program):
  - router logits [1024, 64] loaded as [128p, 8g*64] (host pre-transposes);
    per 128-token group: DVE max/max_index for top-2, ACT exp, DVE reciprocal.
  - hidden stream: 8 tiles of [128, 4096] f32 (2MB), DMA in (SP HWDGE ring),
    per-token scalar multiply (DVE/ACT alternating), DMA out (ACT HWDGE ring).
"""

import numpy as np

N, H, E, K = 8192, 4096, 64, 2
NCORES = 8
NP = N // NCORES  # tokens per core
P = 128           # SBUF partitions
G = NP // P       # 128-token groups per core

_CACHE = {}


def _ensure_path():
    import sys
    try:
        import concourse.bass  # noqa: F401
    except ImportError:
        sys.path.insert(0, "/opt/trn_rl_repo")


def _legalize_waits(bir_json: bytes) -> bytes:
    """Split multi-wait instructions for toolchains whose ISA structs encode
    only one sync wait: extra waits move to standalone EventSemaphore
    instructions inserted just before, on the same engine (sem-ge waits are
    monotonic, so waiting earlier in program order is equivalent)."""
    import json as _json

    m = _json.loads(bir_json)
    counter = 0
    for fn in m["functions"]:
        for blk in fn["blocks"]:
            new_instructions = []
            for ins in blk["instructions"]:
                si = ins.get("sync_info")
                waits = (si or {}).get("on_wait") or []
                if len(waits) > 1 and ins.get("opcode") != "EventSemaphore":
                    for w in waits[:-1]:
                        counter += 1
                        new_instructions.append({
                            "debug": ins.get("debug", 0),
                            "engine": ins["engine"],
                            "ins": [], "outs": [],
                            "name": f"W-{counter}-{ins['name']}",
                            "opcode": "EventSemaphore",
                            "sync_info": {"on_update": [], "on_wait": [w]},
                        })
                    si["on_wait"] = [waits[-1]]
                new_instructions.append(ins)
            blk["instructions"] = new_instructions
    return _json.dumps(m).encode()


def _patch_compiler():
    """Route every BIR compile through _legalize_waits (native + axon paths)."""
    from concourse import bass_utils, bass2jax

    if getattr(bass_utils, "_wait_legalizer_installed", False):
        return
    orig = bass_utils.compile_bir_kernel

    def patched(bir_json, tmpdir, neff_name="file.neff"):
        return orig(_legalize_waits(bytes(bir_json)), tmpdir, neff_name)

    bass_utils.compile_bir_kernel = patched
    bass2jax.compile_bir_kernel = patched
    bass_utils._wait_legalizer_installed = True


def _build_program(repeat=1):
    import concourse.bass as bass
    import concourse.mybir as mybir
    import concourse.tile as tile

    f32 = mybir.dt.float32
    i32 = mybir.dt.int32
    u32 = mybir.dt.uint32
    Exp = mybir.ActivationFunctionType.Exp

    nc = bass.Bass(
        "TRN2", target_bir_lowering=False, debug=False, num_devices=NCORES
    )

    hs = nc.dram_tensor("hs_in", [NP, H], f32, kind="ExternalInput").ap()
    # router logits pre-rearranged on host to [128, G*E]: (p, g*E+e) <- token g*128+p
    rp = nc.dram_tensor("rp_in", [P, G * E], f32, kind="ExternalInput").ap()
    out_c = nc.dram_tensor("combined_out", [NP, H], f32, kind="ExternalOutput").ap()
    # small outputs stay in the [128, G*K] on-chip layout; host de-interleaves
    out_ei = nc.dram_tensor("ei_out", [P, G * K], i32, kind="ExternalOutput").ap()
    out_rw = nc.dram_tensor("rw_out", [P, G * K], f32, kind="ExternalOutput").ap()

    with tile.TileContext(nc) as tc:
        for _rep in range(repeat):
            _emit_body(nc, tc, hs, rp, out_c, out_ei, out_rw)
    return nc


def _emit_body(nc, tc, hs, rp, out_c, out_ei, out_rw):
    import concourse.mybir as mybir

    f32 = mybir.dt.float32
    i32 = mybir.dt.int32
    u32 = mybir.dt.uint32
    Exp = mybir.ActivationFunctionType.Exp

    if True:
        with tc.tile_pool(name="router", bufs=1) as rpool, \
             tc.tile_pool(name="big", bufs=6) as hpool:
            r = rpool.tile([P, G * E], f32, tag="r")
            nc.sync.dma_start(out=r[:], in_=rp[:])
            ei_sb = rpool.tile([P, G * K], i32, tag="ei")
            rw_sb = rpool.tile([P, G * K], f32, tag="rw")
            scale = rpool.tile([P, G], f32, tag="scale")
            # per-group top-8 values/indices, group g at columns [g*8, g*8+8)
            mx_all = rpool.tile([P, G * 8], f32, tag="mxall")
            mi_all = rpool.tile([P, G * 8], u32, tag="miall")

            for g in range(G):
                x = r[:, g * E:(g + 1) * E]
                nc.vector.max(out=mx_all[:, g * 8:(g + 1) * 8], in_=x)
                nc.vector.max_index(
                    out=mi_all[:, g * 8:(g + 1) * 8],
                    in_max=mx_all[:, g * 8:(g + 1) * 8],
                    in_values=x,
                )

            # batched over all G groups via strided views
            m1 = mx_all[:, 0:G * 8:8]   # [P, G] top-1 logit per group
            m2 = mx_all[:, 1:G * 8:8]   # [P, G] top-2 logit per group
            d = rpool.tile([P, G], f32, tag="d")
            nc.vector.tensor_sub(out=d[:], in0=m2, in1=m1)          # m2-m1 <= 0
            e2 = rpool.tile([P, G], f32, tag="e2")
            nc.scalar.activation(e2[:], d[:], Exp)                  # exp(m2-m1)
            denom = rpool.tile([P, G], f32, tag="denom")
            nc.vector.tensor_scalar_add(denom[:], e2[:], 1.0)       # 1+e2
            w1 = rw_sb[:, 0:G * K:2]
            w2 = rw_sb[:, 1:G * K:2]
            nc.vector.reciprocal(w1, denom[:])                      # w1 = 1/(1+e2)
            nc.vector.tensor_mul(out=w2, in0=e2[:], in1=w1)         # w2 = e2/(1+e2)
            nc.vector.tensor_add(out=scale[:], in0=w1, in1=w2)
            nc.vector.tensor_copy(out=ei_sb[:, 0:G * K:2], in_=mi_all[:, 0:G * 8:8])
            nc.vector.tensor_copy(out=ei_sb[:, 1:G * K:2], in_=mi_all[:, 1:G * 8:8])

            nc.sync.dma_start(out=out_ei[:], in_=ei_sb[:])
            nc.sync.dma_start(out=out_rw[:], in_=rw_sb[:])

            for t in range(G):
                h = hpool.tile([P, H], f32, tag="h")
                nc.sync.dma_start(out=h[:], in_=hs[t * P:(t + 1) * P, :])
                nc.vector.tensor_scalar_mul(h[:], h[:], scale[:, t:t + 1])
                nc.scalar.dma_start(out=out_c[t * P:(t + 1) * P, :], in_=h[:])


# test.py hooks: set TRACE=True before calling kernel() to capture a profile.
TRACE = False
LAST_RESULTS = None


def kernel(hidden_states, router_probs, top_k=2, **_unused):
    global LAST_RESULTS
    _ensure_path()
    _patch_compiler()
    from concourse import bass_utils

    assert int(top_k) == K, f"kernel hardcodes top_k={K}, got {top_k}"
    hs = np.ascontiguousarray(np.asarray(hidden_states, dtype=np.float32))
    rp = np.ascontiguousarray(np.asarray(router_probs, dtype=np.float32))
    assert hs.shape == (N, H) and rp.shape == (N, E), (hs.shape, rp.shape)

    if "nc" not in _CACHE:
        _CACHE["nc"] = _build_program()
    nc = _CACHE["nc"]

    in_maps = []
    for c in range(NCORES):
        hs_c = hs[c * NP:(c + 1) * NP]
        rp_c = rp[c * NP:(c + 1) * NP].reshape(G, P, E).transpose(1, 0, 2)
        in_maps.append({
            "hs_in": np.ascontiguousarray(hs_c),
            "rp_in": np.ascontiguousarray(rp_c.reshape(P, G * E)),
        })

    res = bass_utils.run_bass_kernel_spmd(
        nc, in_maps, core_ids=list(range(NCORES)), trace=TRACE
    )
    LAST_RESULTS = res

    combined = np.concatenate([res.results[c]["combined_out"] for c in range(NCORES)], axis=0)
    ei = np.concatenate([
        res.results[c]["ei_out"].reshape(P, G, K).transpose(1, 0, 2).reshape(NP, K)
        for c in range(NCORES)
    ], axis=0)
    rw = np.concatenate([
        res.results[c]["rw_out"].reshape(P, G, K).transpose(1, 0, 2).reshape(NP, K)
        for c in range(NCORES)
    ], axis=0)
    return combined, ei.astype(np.int32, copy=False), rw
